# revision 1
# baseline (speedup 1.0000x reference)
"""Trainium2 Bass kernel for cross-attention (cosine-normalized, 8 heads).

Reference computation (full inputs x,y [1,4096,64]):
  q = x@Wq+bq ; k,v = split(y@Wkv+bkv) ; per head (8 heads, dim 8):
  attn = softmax(l2norm(q) @ l2norm(k)^T) ; out = attn@v
  result = concat_heads(out) @ We + be

Sharding: one head per NeuronCore (8 heads / 8 cores), SPMD program with
per-core weight slices. Each core returns resT_h = (out_h @ We_h + be/8)^T
as [64, 4096]; the host sums over cores and transposes. Measured ~217us
per-core HW exec, absmax rel err ~1.8e-3 vs the fp32 reference.

Key device-side choices (measured on TRN2 via ntff profiles):
  - All matmuls are zero-padded to K=128 contraction: K<=32 matmuls run at
    half the PE clock on this hardware and never warm the activity monitor;
    K=128 streams at full 2.4 GHz (216 ns per N=512 matmul). Host pads
    weights with zero rows; q/k operands live in [128, 4096] tiles whose
    rows 8..127 are zeroed once on gpsimd.
  - Biases are folded into matmuls via ones rows (xTe=[x^T;1;0...]), so the
    kernel contains no bias adds. The ones column appended to V produces the
    softmax denominator in the same PE pass as the numerator.
  - Scores are computed transposed ([k-chunk=128 part, q=512 free]) in fp32r
    (full-rate, ~tf32 precision); cosine scores lie in [-1,1] so softmax
    needs no max subtraction. Exp runs on ScalarE over [128, 1536] PSUM
    spans (GROUPS of 3 chunks amortize the ~352-cycle ACTIVATE overhead),
    writing bf16; the second matmul contracts k with bf16 operands at full
    rate. The (q-block, group) sequence is software-pipelined with one group
    of score-matmul lookahead so ScalarE stays saturated (~141us for 134us
    of exp work).
  - l2 norms: per-block squares fused into the projection phase (q on ACT,
    k on DVE), selector matmuls pack per-block column sums into one [8,512]
    psum, sqrt on ACT (table preloaded) + 2-ULP reciprocal on DVE;
    inverse norms replicated across partitions by row DMAs (partition-
    crossing reshape) on the two HWDGE queues; normalize muls are split
    per block, ordered so the first q-block unblocks as early as possible.
  - Tail: denominator row repacked by one DMA, reciprocal, broadcast,
    single normalize mul into an f32r staging tile whose scaled denominator
    row becomes exactly 1.0 = the bias row for the K-padded fp32r output
    projection; per-block output DMAs overlap the projections.
"""

import sys

import numpy as np

for _p in ("/opt/trn_rl_repo",):
    if _p not in sys.path:
        sys.path.insert(0, _p)

from contextlib import ExitStack

import concourse.bass as bass
import concourse.tile as tile
from concourse import bacc, mybir
from concourse.bass import ts
from concourse.bass_utils import run_bass_kernel_spmd

F32 = mybir.dt.float32
F32R = mybir.dt.float32r
BF16 = mybir.dt.bfloat16

HW = 4096          # sequence length
C = 64             # model dim
H = 8              # heads
D = 8              # head dim
CE = C + 1         # +ones row for bias folding
QB = 512           # q block
NQB = HW // QB     # 8
KC = 128           # k chunk
NKC = HW // KC     # 32
GROUPS = [3] * 10 + [2]   # k-chunks per exp/ACT group (32 total)
GMAX = max(GROUPS)
VW = D + 1         # v + ones column

REPL = "dma"        # inv replication: gpsimd partition_broadcast vs row DMAs

_BUILT = None
TRACE = False
LAST_RESULTS = None


def _body(ctx, tc, dram):
    nc = tc.nc
    xTe_d, yTe_d, wqe_d, wke_d, wve_d, webe_d, sel_d, out_d = dram
    U32 = mybir.dt.uint32

    const = ctx.enter_context(tc.tile_pool(name="const", bufs=1))
    expp = ctx.enter_context(tc.tile_pool(name="exps", bufs=4))
    ps_s = ctx.enter_context(tc.tile_pool(name="ps_s", bufs=2, space="PSUM"))
    ps_o = ctx.enter_context(tc.tile_pool(name="ps_o", bufs=2, space="PSUM"))

    # K-padded tiles: rows beyond the live ones are zeroed so every matmul
    # contracts over K=128 (K<=32 matmuls run at half the PE clock - measured).
    # All zeroing on gpsimd to keep the DVE free for the norm chain.
    xTe = const.tile([KC, HW], F32R)   # host-padded: rows CE..127 zero
    yTe = const.tile([KC, HW], F32R)
    sqq = const.tile([KC, HW], F32R)   # q squares scratch / f32r oT staging
    sqk = const.tile([KC, HW], F32R)   # k squares scratch
    qTn = const.tile([KC, HW], BF16)
    kTn = const.tile([KC, HW], BF16)
    vext = const.tile([KC, VW * NKC], BF16)
    nc.gpsimd.memset(sqq[:].bitcast(U32), 0)
    nc.gpsimd.memset(sqk[:].bitcast(U32), 0)
    nc.gpsimd.memset(qTn[:].bitcast(mybir.dt.uint16), 0)
    nc.gpsimd.memset(kTn[:].bitcast(mybir.dt.uint16), 0)
    nc.gpsimd.memset(vext[:], 1.0)

    # preload the sqrt activation table during the DMA phase so the first
    # real sqrt doesn't eat the ~1.3us table switch on the critical chain
    warm = const.tile([1, 1], F32)
    nc.vector.memset(warm[:], 1.0)
    nc.scalar.sqrt(warm[:], warm[:])

    # ---------------- loads ----------------
    wqe = const.tile([KC, D], F32R)
    nc.sync.dma_start(wqe[:], wqe_d)
    wke = const.tile([KC, D], F32R)
    nc.sync.dma_start(wke[:], wke_d)
    wve = const.tile([KC, D], F32R)
    nc.sync.dma_start(wve[:], wve_d)
    webe = const.tile([KC, C], F32R)
    nc.scalar.dma_start(webe[:], webe_d)
    sel = const.tile([KC, D * NQB], F32R)
    nc.scalar.dma_start(sel[:], sel_d)
    dmae = [nc.sync, nc.scalar]
    for j in range(NQB):
        dmae[j % 2].dma_start(xTe[:, ts(j, QB)], xTe_d[:, ts(j, QB)])
        dmae[(j + 1) % 2].dma_start(yTe[:, ts(j, QB)], yTe_d[:, ts(j, QB)])

    # ---------------- q/k projections (transposed layout) ----------------
    # per-block: projection matmul, copy out, and squares straight from PSUM
    # (q path on ACT, k path on DVE - the two chains run in parallel)
    qT = const.tile([D, HW], F32)
    kT = const.tile([D, HW], F32)
    for j in range(NQB):
        ps = ps_o.tile([D, QB], F32, tag="pso", name=f"qp{j}")
        nc.tensor.matmul(ps[:], wqe[:], xTe[:, ts(j, QB)], start=True,
                         stop=True)
        nc.scalar.copy(qT[:, ts(j, QB)], ps[:])
        nc.scalar.square(sqq[0:D, ts(j, QB)], ps[:])
    for b, w3 in enumerate((3, 3, 2)):   # k in 1536/1536/1024-wide slabs
        base = b * 3
        ps = ps_s.tile([D, GMAX * QB], F32, tag="pss", name=f"kp{b}")
        for u in range(w3):
            nc.tensor.matmul(ps[:, ts(u, QB)], wke[:],
                             yTe[:, ts(base + u, QB)], start=True, stop=True)
        sl = ts(0, w3 * QB)
        dstsl = bass.AP.__getitem__  # noqa - readability only
        nc.vector.tensor_copy(kT[:, base * QB:(base + w3) * QB],
                              ps[:, 0:w3 * QB])
        nc.vector.tensor_mul(sqk[0:D, base * QB:(base + w3) * QB],
                             kT[:, base * QB:(base + w3) * QB],
                             kT[:, base * QB:(base + w3) * QB])

    # ---------------- inverse norms (free layout, partition-packed) -------
    ssq_q = const.tile([NQB, QB], F32)
    ssq_k = const.tile([NQB, QB], F32)
    for ssq, sq in ((ssq_q, sqq), (ssq_k, sqk)):
        ps = (ps_o if sq is sqq else ps_s).tile(
            [NQB, QB], F32, tag="pso" if sq is sqq else "pss")
        for j in range(NQB):
            nc.tensor.matmul(ps[:], sel[:, ts(j, D)], sq[:, ts(j, QB)],
                             start=(j == 0), stop=(j == NQB - 1))
        nc.vector.tensor_copy(ssq[:], ps[:])

    # invsqrt = newton(recip_accurate(sqrt(ssq))); separate scratches so the
    # q and k chains interleave
    inv_q = const.tile([NQB, QB], F32)
    inv_k = const.tile([NQB, QB], F32)
    scrq = const.tile([NQB, QB], F32)
    scr2q = const.tile([NQB, QB], F32)
    scrk = const.tile([NQB, QB], F32)
    scr2k = const.tile([NQB, QB], F32)
    for ssq, inv, sa, sb in ((ssq_q, inv_q, scrq, scr2q),
                             (ssq_k, inv_k, scrk, scr2k)):
        nc.scalar.sqrt(sa[:], ssq[:])
        nc.vector.reciprocal_approx_accurate(inv[:], sa[:], sb[:])

    # replicate inv norms to D partitions (direct row DMAs from the packed
    # [NQB, QB] tile, partition-crossing reshape per row); q rows on the sync
    # HW queue, k rows on the gpsimd SW queue so the chains run in parallel
    # and the scalar/ACT queue is never involved. k reuses the dead xTe tile.
    rep = const.tile([VW, HW], F32)
    rep_k = xTe[0:VW, :].bitcast(F32)
    # independent row DMAs: q rows on sync, k rows on scalar (gpsimd SWDGE
    # posts its semaphores only after a multi-us drain - unusable here)
    for p in range(D):
        nc.sync.dma_start(rep[p:p + 1, :], inv_q[:])
        nc.scalar.dma_start(rep_k[p:p + 1, :], inv_k[:])
    # per-block normalize muls, ordered so q-block 0 and all k chunks come
    # first: the main loop's first groups unblock as early as possible
    def _nmul(dst, srct, rp, j):
        nc.vector.tensor_mul(dst[0:D, ts(j, QB)], srct[:, ts(j, QB)],
                             rp[0:D, ts(j, QB)])
    _nmul(qTn, qT, rep[:], 0)
    for j in range(NQB):
        _nmul(kTn, kT, rep_k, j)
    for j in range(1, NQB):
        _nmul(qTn, qT, rep[:], j)

    # ---------------- v prep (row layout, bf16 out, ones col) -------------
    for c4 in range(NKC // 4):
        ps = ps_s.tile([KC, 4 * D], F32, tag="pss", name=f"vp{c4}")
        for u in range(4):
            nc.tensor.matmul(ps[:, ts(u, D)], yTe[:, ts(4 * c4 + u, KC)],
                             wve[:], start=True, stop=True)
        for u in range(4):
            c = 4 * c4 + u
            nc.scalar.copy(vext[:, c * VW:c * VW + D], ps[:, ts(u, D)])

    # ---------------- main attention loop ----------------
    # oTe rows 0-7: unnormalized numerator; row 8: softmax denominator
    # (after scaling by the replicated reciprocal, row 8 becomes den/den = 1,
    # which is exactly the ones-row the output projection needs for be/8).
    oTe = const.tile([VW, HW], F32)
    den8 = const.tile([NQB, QB], F32)

    # flattened (q-block, group) sequence with one group of score-matmul
    # lookahead, so the PE fills the next group's psum while ScalarE exps the
    # current one - including across q-block boundaries.
    seq = []
    for j in range(NQB):
        c = 0
        for g in GROUPS:
            seq.append((j, c, g))
            c += g
    pos = [None] * NQB
    pss = [None] * len(seq)

    def mm1(i):
        j, c, g = seq[i]
        ps = ps_s.tile([KC, GMAX * QB], F32, tag="pss", name=f"pss{i}")
        pss[i] = ps
        for u in range(g):
            nc.tensor.matmul(ps[:, ts(u, QB)], kTn[:, ts(c + u, KC)],
                             qTn[:, ts(j, QB)], start=True, stop=True)

    mm1(0)
    for i, (j, c, g) in enumerate(seq):
        if pos[j] is None:
            pos[j] = ps_o.tile([VW, QB], F32, tag="pso", name=f"po{j}")
        if i + 1 < len(seq):
            mm1(i + 1)
        ps = pss[i]
        es = expp.tile([KC, GMAX * QB], BF16, tag="es")
        nc.scalar.activation(es[:, 0:g * QB], ps[:, 0:g * QB],
                             mybir.ActivationFunctionType.Exp)
        for u in range(g):
            cc = c + u
            nc.tensor.matmul(pos[j][:], vext[:, cc * VW:(cc + 1) * VW],
                             es[:, ts(u, QB)],
                             start=(cc == 0), stop=(cc == NKC - 1))
        pss[i] = None
        if c + g == NKC:
            nc.vector.tensor_copy(oTe[:, ts(j, QB)], pos[j][:])

    # ---------------- normalize + output projection ----------------
    # repack denominator row [1, HW] -> [NQB, QB] via DMA (partition crossing)
    nc.sync.dma_start(den8[:], oTe[D:D + 1, :])
    invd, scr3 = scr2q, scrq  # prologue scratches, dead by now
    nc.vector.reciprocal_approx_accurate(invd[:], den8[:], scr3[:])
    for p in range(VW):
        dmae[p % 2].dma_start(rep[p:p + 1, :], invd[:])
    # normalized oTe staged per block into the f32r sqq tile (rows 9.. still
    # zero) for a single-pass K-padded fp32r output projection
    resT = const.tile([C, HW], F32)
    for j in range(NQB):
        nc.vector.tensor_mul(sqq[0:VW, ts(j, QB)], oTe[:, ts(j, QB)],
                             rep[:, ts(j, QB)])
        ps = ps_s.tile([C, QB], F32, tag="pss")
        nc.tensor.matmul(ps[:], webe[:], sqq[:, ts(j, QB)], start=True,
                         stop=True)
        nc.scalar.copy(resT[:, ts(j, QB)], ps[:])
        dmae[j % 2].dma_start(out_d[:, ts(j, QB)], resT[:, ts(j, QB)])


def _build():
    global _BUILT
    if _BUILT is not None:
        return _BUILT
    nc = bacc.Bacc("TRN2", target_bir_lowering=False, debug=False, num_devices=H)
    xTe_d = nc.dram_tensor("xTe", [KC, HW], F32R, kind="ExternalInput").ap()
    yTe_d = nc.dram_tensor("yTe", [KC, HW], F32R, kind="ExternalInput").ap()
    wqe_d = nc.dram_tensor("wqe", [KC, D], F32R, kind="ExternalInput").ap()
    wke_d = nc.dram_tensor("wke", [KC, D], F32R, kind="ExternalInput").ap()
    wve_d = nc.dram_tensor("wve", [KC, D], F32R, kind="ExternalInput").ap()
    webe_d = nc.dram_tensor("webe", [KC, C], F32R, kind="ExternalInput").ap()
    sel_d = nc.dram_tensor("sel", [KC, D * NQB], F32R, kind="ExternalInput").ap()
    out_d = nc.dram_tensor("resT", [C, HW], F32, kind="ExternalOutput").ap()
    with tile.TileContext(nc) as tc, ExitStack() as ctx:
        _body(ctx, tc, (xTe_d, yTe_d, wqe_d, wke_d, wve_d, webe_d, sel_d,
                        out_d[:]))
    nc.compile()
    _BUILT = nc
    return nc


def make_in_maps(x, y, Wq, bq, Wkv, bkv, We, be):
    x, y, Wq, bq, Wkv, bkv, We, be = (
        np.asarray(a, np.float32) for a in (x, y, Wq, bq, Wkv, bkv, We, be))
    ones = np.ones((1, HW), np.float32)
    zrows = np.zeros((KC - CE, HW), np.float32)
    xTe = np.ascontiguousarray(np.vstack([x[0].T, ones, zrows]))
    yTe = np.ascontiguousarray(np.vstack([y[0].T, ones, zrows]))
    sel = np.zeros((KC, D * NQB), np.float32)
    for j in range(NQB):
        sel[0:D, D * j + j] = 1.0
    zpad = np.zeros((KC - CE, D), np.float32)
    in_maps = []
    for h in range(H):
        sl = slice(h * D, (h + 1) * D)
        slv = slice(C + h * D, C + (h + 1) * D)
        in_maps.append({
            "xTe": xTe,
            "yTe": yTe,
            "wqe": np.ascontiguousarray(
                np.vstack([Wq[:, sl], bq[None, sl], zpad])),
            "wke": np.ascontiguousarray(
                np.vstack([Wkv[:, sl], bkv[None, sl], zpad])),
            "wve": np.ascontiguousarray(
                np.vstack([Wkv[:, slv], bkv[None, slv], zpad])),
            "webe": np.ascontiguousarray(np.vstack(
                [We[sl, :], be[None, :] / H, np.zeros((KC - VW, C), np.float32)])),
            "sel": sel,
        })
    return in_maps


def kernel(x, y, Wq, bq, Wkv, bkv, We, be):
    global LAST_RESULTS
    nc = _build()
    in_maps = make_in_maps(x, y, Wq, bq, Wkv, bkv, We, be)
    res = run_bass_kernel_spmd(nc, in_maps, core_ids=list(range(H)), trace=TRACE)
    LAST_RESULTS = res
    acc = np.zeros((C, HW), np.float64)
    for r in res.results:
        acc += r["resT"]
    return np.ascontiguousarray(acc.T[None]).astype(np.float32)



# revision 15
# speedup vs baseline: 1.1000x; 1.1000x over previous
"""Trainium2 Bass kernel for cross-attention (cosine-normalized, 8 heads).

Reference computation (full inputs x,y [1,4096,64]):
  q = x@Wq+bq ; k,v = split(y@Wkv+bkv) ; per head (8 heads, dim 8):
  attn = softmax(l2norm(q) @ l2norm(k)^T) ; out = attn@v
  result = concat_heads(out) @ We + be

Sharding: one head per NeuronCore (8 heads / 8 cores), SPMD program with
per-core weight slices. Each core returns resT_h = (out_h @ We_h + be/8)^T
as [64, 4096]; the host sums over cores and transposes.

v2 restructure (baseline was 217.8us; exp stream itself is ~132us of ACT
work and near-irreducible, so this version attacks the 52.5us prologue
and 23us epilogue that were serialized around it):
  - fp16 activations/weights end-to-end: halves input DMA bytes, enables
    2x DVE perf modes, ~8x better elementwise precision than bf16.
  - Inputs DMA'd in quarters on BOTH HWDGE queues (x on scalar, y on
    sync) so per-quarter projection pipelines start as data lands; all
    weights arrive in one packed [128,152] DMA.
  - Prologue engine balance: ACT does q/k psum copies + v copies (ScalarE
    is fastest at PSUM reads) then rsqrts; DVE does the f16 squares +
    one k copy + normalize muls. Exp table is warmed right after the
    rsqrts so the first EXP pays no table load.
  - Norms: selector matmuls (K=8) pack per-block sums into partitions,
    one Rsqrt per side, one stride-0 broadcast DMA per side, f16 2x
    normalize muls (block 0 first to unblock the main loop).
  - Epilogue pipelined INTO the main loop: when block j finishes
    accumulating, its reciprocal/broadcast/normalize/projection/store
    chain drips one stage per (q-block, group) entry under block j+1's
    exp stream. Only block 7's ~5us chain trails the last exp.
"""

import sys

import numpy as np

for _p in ("/opt/trn_rl_repo",):
    if _p not in sys.path:
        sys.path.insert(0, _p)

from contextlib import ExitStack

import concourse.bass as bass
import concourse.tile as tile
from concourse import bacc, mybir
from concourse.bass import ts
from concourse.bass_utils import run_bass_kernel_spmd

F32 = mybir.dt.float32
F16 = mybir.dt.float16
AF = mybir.ActivationFunctionType

HW = 4096          # sequence length
C = 64             # model dim
H = 8              # heads
D = 8              # head dim
QB = 512           # q block
NQB = HW // QB     # 8
KC = 128           # k chunk
NKC = HW // KC     # 32
GROUPS = [3] * 10 + [2]   # k-chunks per exp/ACT group (32 total)
GMAX = max(GROUPS)
VW = D + 1         # v + ones column
QTR = HW // 4      # DMA quarter

# wpack column layout
WQ0, WK0, WV0, WE0, SEL0 = 0, 8, 16, 24, 88
WPC = 152

_BUILT = None
TRACE = False
LAST_RESULTS = None


def _body(ctx, tc, dram):
    nc = tc.nc
    xTe_d, yTe_d, wpack_d, out_d = dram
    U16 = mybir.dt.uint16

    const = ctx.enter_context(tc.tile_pool(name="const", bufs=1))
    expp = ctx.enter_context(tc.tile_pool(name="exps", bufs=4))
    ps_s = ctx.enter_context(tc.tile_pool(name="ps_s", bufs=2, space="PSUM"))
    ps_o = ctx.enter_context(tc.tile_pool(name="ps_o", bufs=2, space="PSUM"))

    xTe = const.tile([KC, HW], F16)    # host: x^T rows 0-63, ones row 64
    yTe = const.tile([KC, HW], F16)
    wpack = const.tile([KC, WPC], F16)
    qT = const.tile([D, HW], F16)
    kT = const.tile([D, HW], F16)
    sqq = const.tile([KC, HW], F16)    # q squares scratch / proj staging
    sqk = const.tile([D, HW], F16)
    qTn = const.tile([KC, HW], F16)
    kTn = const.tile([KC, HW], F16)
    vext = const.tile([KC, VW * NKC], F16)
    inv_q = const.tile([D, QB], F16)
    inv_k = const.tile([D, QB], F16)
    rep_q = const.tile([D, HW], F16)
    rep_k = xTe[0:D, :]                # reuse: x rows dead after q proj
    oTe = const.tile([VW, HW], F32)
    deno = const.tile([1, QB], F32)    # per-block denom row at partition 0
    rcpo = const.tile([1, QB], F32)    # its reciprocal
    invr = const.tile([VW, QB], F32)   # prologue sqrt scratch
    repE = const.tile([D, HW], F32)    # epilogue inv-den broadcast
    resT = const.tile([C, HW], F32)

    # zero padded rows once (gpsimd; overlaps startup + DMA-in). kTn rows
    # >=D must be true zeros (stationary in score matmuls); qTn/sqq pads
    # must at least be finite.
    nc.gpsimd.memset(qTn[:].bitcast(U16), 0)
    nc.gpsimd.memset(kTn[:].bitcast(U16), 0)
    nc.gpsimd.memset(sqq[:].bitcast(U16), 0)
    nc.gpsimd.memset(vext[:], 1.0)           # ones col -> softmax denom

    # preload the sqrt activation table during the DMA phase
    warm = const.tile([1, 1], F32)
    nc.vector.memset(warm[:], 1.0)
    nc.scalar.sqrt(warm[:], warm[:])

    # ---------------- input DMAs: x on scalar queue, y on sync ----------
    nc.sync.dma_start(wpack[:], wpack_d)
    for qq in range(4):
        nc.scalar.dma_start(xTe[:, ts(qq, QTR)], xTe_d[:, ts(qq, QTR)])
        nc.sync.dma_start(yTe[:, ts(qq, QTR)], yTe_d[:, ts(qq, QTR)])

    wq = wpack[:, WQ0:WQ0 + D]
    wk = wpack[:, WK0:WK0 + D]
    wv = wpack[:, WV0:WV0 + D]
    webe = wpack[:, WE0:WE0 + C]

    # ------------- per-quarter projection pipelines (q, k, v) -----------
    # psum copies on ACT (ScalarE has the fastest PSUM path) except the
    # last k quarter on DVE; f16 squares on DVE at 2x.
    vex3 = vext[:].rearrange("p (c v) -> p c v", v=VW)
    for qq in range(4):
        sl = ts(qq, QTR)
        qp = ps_s.tile([KC, GMAX * QB], F32, tag="pss", name=f"qp{qq}")
        nc.tensor.matmul(qp[0:D, 0:QB], wq, xTe[:, ts(2 * qq, QB)],
                         start=True, stop=True)
        nc.tensor.matmul(qp[0:D, QB:QTR], wq, xTe[:, ts(2 * qq + 1, QB)],
                         start=True, stop=True)
        kp = ps_s.tile([KC, GMAX * QB], F32, tag="pss", name=f"kp{qq}")
        nc.tensor.matmul(kp[0:D, 0:QB], wk, yTe[:, ts(2 * qq, QB)],
                         start=True, stop=True)
        nc.tensor.matmul(kp[0:D, QB:QTR], wk, yTe[:, ts(2 * qq + 1, QB)],
                         start=True, stop=True)
        vp = ps_s.tile([KC, GMAX * QB], F32, tag="pss", name=f"vp{qq}")
        for u in range(8):
            c = 8 * qq + u
            nc.tensor.matmul(vp[:, ts(u, D)], yTe[:, ts(c, KC)], wv,
                             start=True, stop=True)
        # copies: ACT (q, k quarters 0-2, v), DVE (k quarter 3)
        nc.scalar.activation(qT[:, sl], qp[0:D, 0:QTR], AF.Copy)
        if qq < 3:
            nc.scalar.activation(kT[:, sl], kp[0:D, 0:QTR], AF.Copy)
        else:
            nc.vector.tensor_copy(kT[:, sl], kp[0:D, 0:QTR])
        vp3 = vp[:, 0:8 * D].rearrange("p (c v) -> p c v", v=D)
        nc.scalar.activation(vex3[:, 8 * qq:8 * qq + 8, 0:D], vp3, AF.Copy)
        # f16 squares (DVE 2x)
        nc.vector.tensor_mul(sqq[0:D, sl], qT[:, sl], qT[:, sl])
        nc.vector.tensor_mul(sqk[:, sl], kT[:, sl], kT[:, sl])

    # ---------------- inverse norms ----------------
    # selector matmuls (K=8) pack per-block column sums into partitions
    ssq_k = ps_o.tile([D, QB], F32, tag="pso", name="ssq_k")
    ssq_q = ps_s.tile([KC, GMAX * QB], F32, tag="pss", name="ssq_q")
    for ssq, sq in ((ssq_k[:], sqk[:]), (ssq_q[0:D, 0:QB], sqq[0:D, :])):
        for j in range(NQB):
            sel_j = wpack[0:D, SEL0 + D * j:SEL0 + D * (j + 1)]
            nc.tensor.matmul(ssq, sel_j, sq[:, ts(j, QB)],
                             start=(j == 0), stop=(j == NQB - 1))
    # inv = 1/sqrt(ssq): ACT sqrt -> DVE fast reciprocal (fp32) -> f16
    # (Rsqrt activation is blocked in bass for accuracy); scratches reuse
    # prologue-dead f32 tiles.
    nc.scalar.sqrt(invr[0:D, :], ssq_k[:])
    nc.scalar.sqrt(oTe[0:D, 0:QB], ssq_q[0:D, 0:QB])
    # exp table load overlaps the recip + bcast + nmul window
    nc.scalar.activation(warm[:], warm[:], AF.Exp)
    nc.vector.reciprocal_approx_fast(repE[:, 0:QB], invr[0:D, :])
    nc.vector.tensor_copy(inv_k[:], repE[:, 0:QB])
    nc.vector.reciprocal_approx_fast(repE[:, QB:2 * QB], oTe[0:D, 0:QB])
    nc.vector.tensor_copy(inv_q[:], repE[:, QB:2 * QB])

    # replicate inverse norms to D partitions via a DRAM bounce: SBUF
    # sources cannot have a stride-0 partition dim, DRAM sources can, so
    # one write + one broadcast-read replaces 8 row DMAs per side.
    dscr_k, _ = tc.tile([D, QB], F16, space="DRAM", name="dscr_k")
    dscr_q, _ = tc.tile([D, QB], F16, space="DRAM", name="dscr_q")
    nc.sync.dma_start(dscr_k[:], inv_k[:])
    nc.scalar.dma_start(dscr_q[:], inv_q[:])
    nc.sync.dma_start(rep_k,
                      dscr_k[:].unsqueeze(0).to_broadcast((D, D, QB)))
    nc.scalar.dma_start(rep_q[:],
                        dscr_q[:].unsqueeze(0).to_broadcast((D, D, QB)))
    # sqq ones row (for the be/8 bias fold) copied from xTe's host ones
    # row — engine memsets cannot start at partition 8
    nc.sync.dma_start(sqq[D:D + 1, :], xTe[C:C + 1, :])

    # normalize muls (f16 2x): block 0 first to unblock the main loop
    nc.vector.tensor_mul(kTn[0:D, 0:QB], kT[:, 0:QB], rep_k[:, 0:QB])
    nc.vector.tensor_mul(qTn[0:D, 0:QB], qT[:, 0:QB], rep_q[:, 0:QB])
    nc.vector.tensor_mul(kTn[0:D, QB:], kT[:, QB:], rep_k[:, QB:])
    nc.vector.tensor_mul(qTn[0:D, QB:], qT[:, QB:], rep_q[:, QB:])

    # ---------------- main attention loop + dripped epilogue ------------
    dscr_e, _ = tc.tile([NQB, QB], F32, space="DRAM", name="dscr_e")
    seq = []
    for j in range(NQB):
        c = 0
        for g in GROUPS:
            seq.append((j, c, g))
            c += g
    pos = [None] * NQB
    pss = [None] * len(seq)

    def mm1(i):
        j, c, g = seq[i]
        ps = ps_s.tile([KC, GMAX * QB], F32, tag="pss", name=f"pss{i}")
        pss[i] = ps
        for u in range(g):
            nc.tensor.matmul(ps[:, ts(u, QB)], kTn[:, ts(c + u, KC)],
                             qTn[:, ts(j, QB)], start=True, stop=True)

    def epi_stages(j):
        """Per-block epilogue: 1/den, broadcast, normalize, project,
        store — dripped one stage per subsequent loop entry."""
        projps = [None]

        def s_den(j=j):
            # engine ops need quadrant-aligned partition bases, so the
            # denom row (partition 8) moves to partition 0 by DMA first
            nc.sync.dma_start(deno[:], oTe[D:D + 1, ts(j, QB)])

        def s_recip(j=j):
            nc.vector.reciprocal_approx_fast(rcpo[:], deno[:])

        def s_wr(j=j):
            nc.sync.dma_start(dscr_e[j:j + 1, :], rcpo[:])

        def s_bcast(j=j):
            nc.sync.dma_start(repE[:, ts(j, QB)],
                              dscr_e[j:j + 1, :].to_broadcast((D, QB)))

        def s_mul(j=j):
            nc.vector.tensor_mul(sqq[0:D, ts(j, QB)], oTe[0:D, ts(j, QB)],
                                 repE[:, ts(j, QB)])

        def s_proj(j=j):
            ps = ps_o.tile([C, QB], F32, tag="pso", name=f"proj{j}")
            nc.tensor.matmul(ps[:], webe, sqq[:, ts(j, QB)], start=True,
                             stop=True)
            projps[0] = ps

        def s_copy(j=j):
            nc.vector.tensor_copy(resT[:, ts(j, QB)], projps[0][:])

        def s_out(j=j):
            nc.sync.dma_start(out_d[:, ts(j, QB)], resT[:, ts(j, QB)])

        return [s_den, s_recip, s_wr, s_bcast, s_mul, s_proj, s_copy, s_out]

    pend = []

    mm1(0)
    for i, (j, c, g) in enumerate(seq):
        if pos[j] is None:
            pos[j] = ps_o.tile([VW, QB], F32, tag="pso", name=f"po{j}")
        if i + 1 < len(seq):
            mm1(i + 1)
        ps = pss[i]
        es = expp.tile([KC, GMAX * QB], F16, tag="es")
        nc.scalar.activation(es[:, 0:g * QB], ps[:, 0:g * QB], AF.Exp)
        for u in range(g):
            cc = c + u
            nc.tensor.matmul(pos[j][:], vext[:, cc * VW:(cc + 1) * VW],
                             es[:, ts(u, QB)],
                             start=(cc == 0), stop=(cc == NKC - 1))
        pss[i] = None
        if c + g == NKC:
            nc.vector.tensor_copy(oTe[:, ts(j, QB)], pos[j][:])
            pend.append(epi_stages(j))
        elif pend:
            pend[0].pop(0)()
            if not pend[0]:
                pend.pop(0)
    while pend:
        pend[0].pop(0)()
        if not pend[0]:
            pend.pop(0)


def _build():
    global _BUILT
    if _BUILT is not None:
        return _BUILT
    nc = bacc.Bacc("TRN2", target_bir_lowering=False, debug=False, num_devices=H)
    xTe_d = nc.dram_tensor("xTe", [KC, HW], F16, kind="ExternalInput").ap()
    yTe_d = nc.dram_tensor("yTe", [KC, HW], F16, kind="ExternalInput").ap()
    wpack_d = nc.dram_tensor("wpack", [KC, WPC], F16, kind="ExternalInput").ap()
    out_d = nc.dram_tensor("resT", [C, HW], F32, kind="ExternalOutput").ap()
    with tile.TileContext(nc) as tc, ExitStack() as ctx:
        _body(ctx, tc, (xTe_d, yTe_d, wpack_d, out_d[:]))
    nc.compile()
    _BUILT = nc
    return nc


def make_in_maps(x, y, Wq, bq, Wkv, bkv, We, be):
    x, y, Wq, bq, Wkv, bkv, We, be = (
        np.asarray(a, np.float32) for a in (x, y, Wq, bq, Wkv, bkv, We, be))
    ones = np.ones((1, HW), np.float32)
    zrows = np.zeros((KC - C - 1, HW), np.float32)
    xTe = np.vstack([x[0].T, ones, zrows]).astype(np.float16)
    yTe = np.vstack([y[0].T, ones, zrows]).astype(np.float16)
    sel = np.zeros((KC, C), np.float32)
    for j in range(NQB):
        sel[0:D, D * j + j] = 1.0
    in_maps = []
    for h in range(H):
        sl = slice(h * D, (h + 1) * D)
        slv = slice(C + h * D, C + (h + 1) * D)
        zc = np.zeros((KC - C - 1, D), np.float32)
        wqe = np.vstack([Wq[:, sl], bq[None, sl], zc])
        wke = np.vstack([Wkv[:, sl], bkv[None, sl], zc])
        wve = np.vstack([Wkv[:, slv], bkv[None, slv], zc])
        webe = np.vstack([We[sl, :], be[None, :] / H,
                          np.zeros((KC - VW, C), np.float32)])
        wpack = np.concatenate([wqe, wke, wve, webe, sel], axis=1)
        in_maps.append({
            "xTe": xTe,
            "yTe": yTe,
            "wpack": np.ascontiguousarray(wpack.astype(np.float16)),
        })
    return in_maps


def kernel(x, y, Wq, bq, Wkv, bkv, We, be):
    global LAST_RESULTS
    nc = _build()
    in_maps = make_in_maps(x, y, Wq, bq, Wkv, bkv, We, be)
    res = run_bass_kernel_spmd(nc, in_maps, core_ids=list(range(H)), trace=TRACE)
    LAST_RESULTS = res
    acc = np.zeros((C, HW), np.float64)
    for r in res.results:
        acc += r["resT"]
    return np.ascontiguousarray(acc.T[None]).astype(np.float32)
